# revision 21
# baseline (speedup 1.0000x reference)
"""Causal multi-head self-attention with RoPE on 8 Trainium2 NeuronCores.

Sharding: batch (2) x head-groups (4 heads each) -> 8 cores.
Core c: batch b = c // 4, heads 4*(c%4) .. 4*(c%4)+3.
Each core computes Q/K/V projections for its head shard over the full
sequence, RoPE, causal attention (scores computed transposed, softmax via
an appended ones-block in the attn@V matmul), and its partial output
projection.  The 4 partial outputs per batch are summed on the host
(the all-reduce step of the head/tensor-parallel layout).

All matmul operands are bf16 (FWL weight loads, half the DMA traffic);
PSUM accumulation stays fp32.  Exp is restricted to the causal region;
masked sub-blocks are zeroed on GpSimd.  V projection overlaps the first
query-chunk's score/exp stream.

Self-contained: hardcodes all shapes; builds/compiles the Bass program on
first call and caches it.
"""

import numpy as np
import ml_dtypes

import concourse.bass as bass
import concourse.mybir as mybir
import concourse.tile as tile
from concourse import bacc
from concourse.bass import ts, ds
from concourse.bass_utils import run_bass_kernel_spmd

# Problem shape (fixed)
B = 2
S = 2048
D_MODEL = 1024
N_HEADS = 16
D_K = 64
ROPE_THETA = 10000.0

N_CORES = 8
HEADS_PER_CORE = 4
HD = HEADS_PER_CORE * D_K           # 256 head features per core
P = 128
QC = 512                             # query chunk (free dim of S^T tiles)
N_QC = S // QC                       # 4
N_KC = S // P                        # 16 key chunks
KD = D_MODEL // P                    # 8 contraction chunks for projections

F32 = mybir.dt.float32
BF16 = mybir.dt.bfloat16
EXP = mybir.ActivationFunctionType.Exp

BF16_NP = ml_dtypes.bfloat16


def build_nc(dump=False):
    nc = bacc.Bacc("TRN2", target_bir_lowering=False, debug=False,
                   num_devices=N_CORES)

    # DRAM I/O (per-core shards, same names on every core)
    xT = nc.dram_tensor("xT", [D_MODEL, S], BF16, kind="ExternalInput")
    wqT = nc.dram_tensor("wqT", [D_MODEL, HD], BF16, kind="ExternalInput")
    wkT = nc.dram_tensor("wkT", [D_MODEL, HD], BF16, kind="ExternalInput")
    wvT = nc.dram_tensor("wvT", [D_MODEL, HD], BF16, kind="ExternalInput")
    woT = nc.dram_tensor("woT", [HD, D_MODEL], BF16, kind="ExternalInput")
    cosT = nc.dram_tensor("cosT", [P, S], BF16, kind="ExternalInput")
    sinT = nc.dram_tensor("sinT", [P, S], BF16, kind="ExternalInput")
    maskT = nc.dram_tensor("maskT", [P, P], BF16, kind="ExternalInput")
    out = nc.dram_tensor("out", [S, D_MODEL], BF16, kind="ExternalOutput")

    if dump:
        dbg_qt = nc.dram_tensor("dbg_qt", [P, 2, S], BF16,
                                kind="ExternalOutput")
        dbg_kt = nc.dram_tensor("dbg_kt", [P, 2, S], BF16,
                                kind="ExternalOutput")
        dbg_vo = nc.dram_tensor("dbg_vo", [P, N_KC, HEADS_PER_CORE, P],
                                BF16, kind="ExternalOutput")
        dbg_att = nc.dram_tensor("dbg_att", [P, 2, S], BF16,
                                 kind="ExternalOutput")
        dbg_e2 = nc.dram_tensor("dbg_e2", [P, 2, 2, QC], BF16,
                                kind="ExternalOutput")

    xT_r = xT.ap().rearrange("(o p) s -> p o s", p=P)
    wq_r = wqT.ap().rearrange("(o p) f -> p o f", p=P)
    wk_r = wkT.ap().rearrange("(o p) f -> p o f", p=P)
    wv_r = wvT.ap().rearrange("(o p) f -> p o f", p=P)

    with tile.TileContext(nc) as tc, \
            tc.tile_pool(name="res", bufs=1) as res, \
            tc.tile_pool(name="qkv", bufs=1) as qkvp:
        qt = res.tile([P, 2, S], BF16)               # Q^T rotated
        kt = res.tile([P, 2, S], BF16)               # K^T rotated
        # V with ones block: sub=0 heads [V | 1], sub=1 heads [1 | V]
        vo = res.tile([P, N_KC, HEADS_PER_CORE, P], BF16)
        att = res.tile([P, 2, S], BF16)              # normalized attn^T
        wo_s = res.tile([P, 2, D_MODEL], BF16)
        mask_s = res.tile([P, P], BF16)

        # ---- load inputs; per-chunk DMAs so compute starts early ----
        xTs = qkvp.tile([P, KD, S], BF16)            # 32 KB/part
        wq_s = qkvp.tile([P, KD, HD], BF16)
        wk_s = qkvp.tile([P, KD, HD], BF16)
        wv_s = qkvp.tile([P, KD, HD], BF16)
        for kc in range(KD):
            nc.sync.dma_start(wq_s[:, kc, :], wq_r[:, kc, :])
            nc.sync.dma_start(xTs[:, kc, :], xT_r[:, kc, :])
            nc.sync.dma_start(wk_s[:, kc, :], wk_r[:, kc, :])
            nc.sync.dma_start(wv_s[:, kc, :], wv_r[:, kc, :])
        cos_s = qkvp.tile([P, S], BF16)
        nc.sync.dma_start(cos_s[:], cosT.ap())
        sin_s = qkvp.tile([P, S], BF16)
        nc.sync.dma_start(sin_s[:], sinT.ap())
        nc.sync.dma_start(wo_s[:], woT.ap().rearrange("(o p) n -> p o n", p=P))
        nc.sync.dma_start(mask_s[:], maskT.ap())
        # vo ones filled once; V projection overwrites the V columns
        nc.gpsimd.memset(vo[:], 1.0)

        with tc.tile_pool(name="ppj", bufs=1, space="PSUM") as ppj:
            # PE warm-up: dummy bf16 matmuls while input DMAs stream, so the
            # HAM clock gate opens before the first real matmul.
            warm = qkvp.tile([P, 640], BF16)
            nc.vector.memset(warm[:], 0.5)
            wscr = nc.dram_tensor("warm_scratch", [1, 4], F32)
            wps = ppj.tile([P, QC], F32, tag="pp", bufs=8)
            NWARM = 9
            for _w in range(NWARM):
                nc.tensor.matmul(wps[:], warm[:, 0:P], warm[:, P:P + QC],
                                 start=(_w == 0), stop=(_w == NWARM - 1))
            wkeep = qkvp.tile([1, 4], F32)
            nc.vector.tensor_copy(wkeep[:], wps[0:1, 0:4])
            # preload the Exp activation table while the PE warms up, so the
            # first real exp of the attention phase doesn't pay the table
            # load on the critical path
            nc.scalar.activation(out=wkeep[:], in_=wkeep[:], func=EXP,
                                 scale=0.0)
            nc.sync.dma_start(wscr.ap(), wkeep[:])

            # ---- Q/K projections + RoPE.  PSUM is first copied to bf16 by
            # the (otherwise idle) ScalarE so the DVE muls run at 2x rate;
            # the 32-row partition swap goes through SBUF-SBUF DMA (engines
            # cannot cross partitions between two SBUF operands).
            with tc.tile_pool(name="rope", bufs=2) as rope:
                def rope_drain(pp0, pp1, dst, nt):
                    # drain both pairs of one seq chunk together: halves the
                    # op count, and the 4 swap DMAs move [32,2,512] each
                    # (2 KB/partition contiguous runs) split over two queues
                    qraw = rope.tile([P, 2, QC], BF16, tag="qr")
                    nc.scalar.copy(qraw[:, 0, :], pp0[:])
                    nc.scalar.copy(qraw[:, 1, :], pp1[:])
                    t1 = rope.tile([P, 2, QC], BF16, tag="t1")
                    nc.vector.tensor_mul(
                        t1[:], qraw[:],
                        cos_s[:, None, ts(nt, QC)].to_broadcast((P, 2, QC)))
                    y = rope.tile([P, 2, QC], BF16, tag="y")
                    nc.vector.tensor_mul(
                        y[:], qraw[:],
                        sin_s[:, None, ts(nt, QC)].to_broadcast((P, 2, QC)))
                    t2 = rope.tile([P, 2, QC], BF16, tag="t2")
                    for blk in range(4):
                        sb = blk ^ 1
                        eng = nc.sync if blk % 2 == 0 else nc.gpsimd
                        eng.dma_start(t2[ts(blk, 32), :, :],
                                      y[ts(sb, 32), :, :])
                    nc.vector.tensor_add(dst[:, :, ts(nt, QC)],
                                         t1[:], t2[:])

                # Q: kc-outer so compute starts as x chunks stream in
                pps = {}
                for nt in range(N_QC):
                    for pair in range(2):
                        pps[(pair, nt)] = ppj.tile(
                            [P, QC], F32, tag="pp", bufs=8,
                            name=f"pp0_{pair}_{nt}")
                for kc in range(KD):
                    for nt in range(N_QC):
                        for pair in range(2):
                            nc.tensor.matmul(
                                pps[(pair, nt)][:],
                                wq_s[:, kc, ts(pair, P)],
                                xTs[:, kc, ts(nt, QC)],
                                start=(kc == 0), stop=(kc == KD - 1),
                            )
                for nt in range(N_QC):
                    rope_drain(pps[(0, nt)], pps[(1, nt)], qt, nt)
                # K: nt-outer (x now resident) so PSUM banks are consumed
                # incrementally, chasing Q's drains
                for nt in range(N_QC):
                    ppk = {}
                    for pair in range(2):
                        ppk[pair] = ppj.tile([P, QC], F32, tag="pp", bufs=8,
                                             name=f"pp1_{pair}_{nt}")
                        for kc in range(KD):
                            nc.tensor.matmul(
                                ppk[pair][:],
                                wk_s[:, kc, ts(pair, P)],
                                xTs[:, kc, ts(nt, QC)],
                                start=(kc == 0), stop=(kc == KD - 1),
                            )
                    rope_drain(ppk[0], ppk[1], kt, nt)

        if dump:
            nc.sync.dma_start(dbg_qt.ap(), qt[:])
            nc.sync.dma_start(dbg_kt.ap(), kt[:])

        with (
            tc.tile_pool(name="att_sb", bufs=8) as esb,
            tc.tile_pool(name="att_misc", bufs=4) as misc,
            tc.tile_pool(name="pst", bufs=2, space="PSUM") as pst,
            tc.tile_pool(name="out_sb", bufs=3) as outp,
        ):
            es_g = {}
            scored = set()

            def do_score(qcv, kc):
                # scores^T for 4 heads: per pair one K=64 matmul per sub,
                # subs run concurrently in row-groups (0,*) / (64,*)
                e2 = esb.tile([P, 2, 2, QC], BF16, tag="e",
                              name=f"e{qcv}_{kc}")
                r = kc - 4 * qcv
                q0 = max(r, 0) * P          # first unmasked query column
                for pair in range(2):
                    st2 = pst.tile([P, 2, QC], F32, tag="st",
                                   name=f"st{qcv}_{kc}_{pair}")
                    for sub in range(2):
                        nc.tensor.matmul(
                            st2[:, sub, :],
                            kt[ts(sub, D_K), pair, ts(kc, P)],
                            qt[ts(sub, D_K), pair, ts(qcv, QC)],
                            start=True, stop=True,
                        )
                    if q0 > 0:
                        nc.gpsimd.memset(e2[:, pair, :, 0:q0], 0.0)
                        nc.scalar.activation(out=e2[:, pair, :, q0:QC],
                                             in_=st2[:, :, q0:QC], func=EXP,
                                             scale=0.125)
                    else:
                        nc.scalar.activation(out=e2[:, pair, :, :],
                                             in_=st2[:], func=EXP,
                                             scale=0.125)
                if r >= 0:
                    # zero the masked upper half of the diagonal 128-block
                    nc.vector.tensor_mul(
                        e2[:, :, :, ds(q0, P)], e2[:, :, :, ds(q0, P)],
                        mask_s[:, None, None, :].to_broadcast((P, 2, 2, P)))
                if dump and qcv == 0 and kc == 0:
                    nc.sync.dma_start(dbg_e2.ap(), e2[:])
                es_g[(qcv, kc)] = e2
                scored.add((qcv, kc))

            def emit_wo_sc(sc, last=False):
                # output projection for one 128-row s-chunk
                ot = outp.tile([P, D_MODEL], BF16, tag="ot", name=f"ot{sc}")
                for n2 in range(2):
                    po = pst.tile([P, 2, QC], F32, tag="st",
                                  name=f"po{sc}_{n2}")[:, 0, :]
                    for pair in range(2):
                        nc.tensor.matmul(
                            po[:],
                            att[:, pair, ts(sc, P)],
                            wo_s[:, pair, ts(n2, QC)],
                            start=(pair == 0), stop=(pair == 1),
                        )
                    if last:
                        # tail: DVE is busy normalizing; ScalarE is idle
                        nc.scalar.copy(ot[:, ts(n2, QC)], po[:])
                    else:
                        nc.vector.tensor_copy(ot[:, ts(n2, QC)], po[:])
                nc.sync.dma_start(out.ap()[ts(sc, P), :], ot[:])

            # ---- V projection into [V|1] / [1|V] layout, overlapped with
            # the first query-chunk's scores+exp stream ----
            with tc.tile_pool(name="pvp", bufs=3, space="PSUM") as pvp:
                for sc in range(N_KC):
                    pv = pvp.tile([P, HD], F32, tag="pv")
                    for kc in range(KD):
                        nc.tensor.matmul(
                            pv[:],
                            xTs[:, kc, ts(sc, P)],
                            wv_s[:, kc, :],
                            start=(kc == 0), stop=(kc == KD - 1),
                        )
                    # heads (0,2) V at cols 0:64; heads (1,3) at 64:128
                    nc.scalar.copy(
                        vo[:, sc, 0:HEADS_PER_CORE:2, 0:D_K],
                        pv[:].rearrange("p (h d) -> p h d", d=D_K)[:, 0::2, :])
                    nc.scalar.copy(
                        vo[:, sc, 1:HEADS_PER_CORE:2, D_K:P],
                        pv[:].rearrange("p (h d) -> p h d", d=D_K)[:, 1::2, :])
                    if sc == 3:
                        # scores for qc0 can start as soon as qt/kt are
                        # rotated; they only need pst banks, not pat
                        for kc2 in range(4):
                            do_score(0, kc2)
                if dump:
                    nc.sync.dma_start(dbg_vo.ap(), vo[:])

            # ---- attention + interleaved output projection ----
            with tc.tile_pool(name="pat", bufs=4, space="PSUM") as pat:
                pending_wo = None
                for qc in range(N_QC):
                    pas = {}
                    n_kc = 4 * qc + 4
                    for pair in range(2):
                        for sub in range(2):
                            pas[(pair, sub)] = pat.tile(
                                [P, QC], F32, tag="pa",
                                name=f"pa{qc}{pair}{sub}")
                    def attnv(kc_v, pas=pas, n_kc=n_kc, qc=qc):
                        e2 = es_g.pop((qc, kc_v))
                        for pair in range(2):
                            for sub in range(2):
                                h = pair * 2 + sub
                                nc.tensor.matmul(
                                    pas[(pair, sub)][:],
                                    vo[:, kc_v, h, :],
                                    e2[:, pair, sub, :],
                                    start=(kc_v == 0), stop=(kc_v == n_kc - 1),
                                )

                    KB = 3
                    done_v = 0
                    for kc in range(n_kc):
                        if (qc, kc) not in scored:
                            do_score(qc, kc)
                        # spread the previous qc's output projection through
                        # this qc's score stream so its po tiles don't burst-
                        # starve the exp pipeline of pst slots
                        if pending_wo is not None and kc < 8 and kc % 2 == 1:
                            emit_wo_sc(4 * pending_wo + kc // 2)
                            if kc == 7:
                                pending_wo = None
                        if kc + 1 - done_v >= 2 * KB:
                            for kc_v in range(done_v, done_v + KB):
                                attnv(kc_v)
                            done_v += KB
                    # prefetch the next qc's first score groups: dense PE
                    # cover for the tail attnv flush
                    if qc + 1 < N_QC:
                        for kc2 in range(4):
                            do_score(qc + 1, kc2)
                    for kc_v in range(done_v, n_kc):
                        attnv(kc_v)

                    # normalize: att = attn_rows * recip(rowsum_rows)
                    for pair in range(2):
                        for sub in range(2):
                            pa = pas[(pair, sub)]
                            rs = misc.tile([P, QC], F32, tag="rb")
                            if sub == 0:
                                rows = slice(64, 128)   # rowsum rows
                                arows = slice(0, 64)    # attn rows
                            else:
                                rows = slice(0, 64)
                                arows = slice(64, 128)
                            # full-partition op: the custom-DVE recip
                            # misbehaves at base_partition 64; unused rows
                            # are harmless
                            nc.vector.reciprocal_approx_fast(rs[:], pa[:])
                            nc.vector.tensor_mul(
                                att[arows, pair, ts(qc, QC)],
                                pa[arows, :], rs[rows, :])
                    pending_wo = qc
                for sc in range(4 * pending_wo, 4 * pending_wo + 4):
                    emit_wo_sc(sc, last=True)
                if dump:
                    nc.sync.dma_start(dbg_att.ap(), att[:])

    nc.compile()
    return nc

_NC_CACHE = {}


def _get_nc(dump=False):
    key = ("dump" if dump else "nc")
    if key not in _NC_CACHE:
        _NC_CACHE[key] = build_nc(dump)
    return _NC_CACHE[key]


def _host_shards(x, token_positions, Wq, Wk, Wv, Wo):
    x = np.asarray(x, dtype=np.float32)
    pos = np.asarray(token_positions).astype(np.float32)
    Wq = np.asarray(Wq, dtype=np.float32)
    Wk = np.asarray(Wk, dtype=np.float32)
    Wv = np.asarray(Wv, dtype=np.float32)
    Wo = np.asarray(Wo, dtype=np.float32)

    # RoPE tables
    j = np.arange(0, D_K, 2, dtype=np.float32) / D_K
    inv_freq = (ROPE_THETA ** (-j)).astype(np.float32)        # [32]
    ang = pos[None, :] * inv_freq[:, None]                    # [32, S]
    cos32 = np.cos(ang).astype(np.float32)
    sin32 = np.sin(ang).astype(np.float32)
    cosT = np.tile(cos32, (4, 1)).astype(BF16_NP)             # [128, S]
    sinT = np.concatenate([sin32, -sin32, sin32, -sin32],
                          axis=0).astype(BF16_NP)

    # causal 128x128 block mask: mask[kp, j] = (j >= kp)
    kp = np.arange(P)[:, None]
    jq = np.arange(P)[None, :]
    mask = (jq >= kp).astype(BF16_NP)

    perm = np.concatenate([np.arange(0, D_K, 2), np.arange(1, D_K, 2)])

    in_maps = []
    for c in range(N_CORES):
        b = c // 4
        hg = c % 4
        heads = np.arange(4 * hg, 4 * hg + 4)
        rows_perm = np.concatenate([h * D_K + perm for h in heads])
        rows = np.concatenate([h * D_K + np.arange(D_K) for h in heads])
        in_maps.append({
            "xT": np.ascontiguousarray(x[b].T).astype(BF16_NP),
            "wqT": np.ascontiguousarray(Wq[rows_perm, :].T).astype(BF16_NP),
            "wkT": np.ascontiguousarray(Wk[rows_perm, :].T).astype(BF16_NP),
            "wvT": np.ascontiguousarray(Wv[rows, :].T).astype(BF16_NP),
            "woT": np.ascontiguousarray(Wo[:, rows].T).astype(BF16_NP),
            "cosT": cosT,
            "sinT": sinT,
            "maskT": mask,
        })
    return in_maps


def kernel(x, token_positions, Wq, Wk, Wv, Wo, use_f32r=True, trace=False):
    nc = _get_nc()
    in_maps = _host_shards(x, token_positions, Wq, Wk, Wv, Wo)
    res = run_bass_kernel_spmd(nc, in_maps, list(range(N_CORES)), trace=trace)
    outs = [np.asarray(res.results[c]["out"], dtype=np.float32)
            for c in range(N_CORES)]
    full = np.empty((B, S, D_MODEL), dtype=np.float32)
    for b in range(B):
        full[b] = outs[4 * b] + outs[4 * b + 1] + outs[4 * b + 2] + outs[4 * b + 3]
    kernel.last_result = res
    return full


# revision 24
# speedup vs baseline: 1.0315x; 1.0315x over previous
"""Causal multi-head self-attention with RoPE on 8 Trainium2 NeuronCores.

Sharding: batch (2) x head-groups (4 heads each) -> 8 cores.
Core c: batch b = c // 4, heads 4*(c%4) .. 4*(c%4)+3.
Each core computes Q/K/V projections for its head shard over the full
sequence, RoPE, causal attention (scores computed transposed, softmax via
an appended ones-block in the attn@V matmul), and its partial output
projection.  The 4 partial outputs per batch are summed on the host
(the all-reduce step of the head/tensor-parallel layout).

All matmul operands are bf16 (FWL weight loads, half the DMA traffic);
PSUM accumulation stays fp32.  Exp is restricted to the causal region;
masked sub-blocks are zeroed on GpSimd.  V projection overlaps the first
query-chunk's score/exp stream.

Self-contained: hardcodes all shapes; builds/compiles the Bass program on
first call and caches it.
"""

import numpy as np
import ml_dtypes

import concourse.bass as bass
import concourse.mybir as mybir
import concourse.tile as tile
from concourse import bacc
from concourse.bass import ts, ds
from concourse.bass_utils import run_bass_kernel_spmd

# Problem shape (fixed)
B = 2
S = 2048
D_MODEL = 1024
N_HEADS = 16
D_K = 64
ROPE_THETA = 10000.0

N_CORES = 8
HEADS_PER_CORE = 4
HD = HEADS_PER_CORE * D_K           # 256 head features per core
P = 128
QC = 512                             # query chunk (free dim of S^T tiles)
N_QC = S // QC                       # 4
N_KC = S // P                        # 16 key chunks
KD = D_MODEL // P                    # 8 contraction chunks for projections

F32 = mybir.dt.float32
BF16 = mybir.dt.bfloat16
EXP = mybir.ActivationFunctionType.Exp

BF16_NP = ml_dtypes.bfloat16


def build_nc(dump=False):
    nc = bacc.Bacc("TRN2", target_bir_lowering=False, debug=False,
                   num_devices=N_CORES)

    # DRAM I/O (per-core shards, same names on every core)
    xT = nc.dram_tensor("xT", [D_MODEL, S], BF16, kind="ExternalInput")
    wqT = nc.dram_tensor("wqT", [D_MODEL, HD], BF16, kind="ExternalInput")
    wkT = nc.dram_tensor("wkT", [D_MODEL, HD], BF16, kind="ExternalInput")
    wvT = nc.dram_tensor("wvT", [D_MODEL, HD], BF16, kind="ExternalInput")
    woT = nc.dram_tensor("woT", [HD, D_MODEL], BF16, kind="ExternalInput")
    cosT = nc.dram_tensor("cosT", [P, S], BF16, kind="ExternalInput")
    sinT = nc.dram_tensor("sinT", [P, S], BF16, kind="ExternalInput")
    maskT = nc.dram_tensor("maskT", [P, P], BF16, kind="ExternalInput")
    out = nc.dram_tensor("out", [S, D_MODEL], BF16, kind="ExternalOutput")

    if dump:
        dbg_qt = nc.dram_tensor("dbg_qt", [P, 2, S], BF16,
                                kind="ExternalOutput")
        dbg_kt = nc.dram_tensor("dbg_kt", [P, 2, S], BF16,
                                kind="ExternalOutput")
        dbg_vo = nc.dram_tensor("dbg_vo", [P, N_KC, HEADS_PER_CORE, P],
                                BF16, kind="ExternalOutput")
        dbg_att = nc.dram_tensor("dbg_att", [P, 2, S], BF16,
                                 kind="ExternalOutput")
        dbg_e2 = nc.dram_tensor("dbg_e2", [P, 2, 2, QC], BF16,
                                kind="ExternalOutput")

    xT_r = xT.ap().rearrange("(o p) s -> p o s", p=P)
    wq_r = wqT.ap().rearrange("(o p) f -> p o f", p=P)
    wk_r = wkT.ap().rearrange("(o p) f -> p o f", p=P)
    wv_r = wvT.ap().rearrange("(o p) f -> p o f", p=P)

    with tile.TileContext(nc) as tc, \
            tc.tile_pool(name="res", bufs=1) as res, \
            tc.tile_pool(name="qkv", bufs=1) as qkvp:
        qt = res.tile([P, 2, S], BF16)               # Q^T rotated
        kt = res.tile([P, 2, S], BF16)               # K^T rotated
        # V with ones block: sub=0 heads [V | 1], sub=1 heads [1 | V]
        vo = res.tile([P, N_KC, HEADS_PER_CORE, P], BF16)
        att = res.tile([P, 2, S], BF16)              # normalized attn^T
        wo_s = res.tile([P, 2, D_MODEL], BF16)
        mask_s = res.tile([P, P], BF16)

        # ---- load inputs; per-chunk DMAs so compute starts early ----
        xTs = qkvp.tile([P, KD, S], BF16)            # 32 KB/part
        wq_s = qkvp.tile([P, KD, HD], BF16)
        wk_s = qkvp.tile([P, KD, HD], BF16)
        wv_s = qkvp.tile([P, KD, HD], BF16)
        # x chunks on the Sync queue alone (the critical path for Q's
        # kc-outer stream); weights and tables on other queues so the
        # ~0.6us-per-DMA issue cost doesn't serialize behind x
        for kc in range(KD):
            nc.sync.dma_start(xTs[:, kc, :], xT_r[:, kc, :])
            nc.gpsimd.dma_start(wq_s[:, kc, :], wq_r[:, kc, :])
            nc.gpsimd.dma_start(wk_s[:, kc, :], wk_r[:, kc, :])
            nc.scalar.dma_start(wv_s[:, kc, :], wv_r[:, kc, :])
        cos_s = qkvp.tile([P, S], BF16)
        nc.scalar.dma_start(cos_s[:], cosT.ap())
        sin_s = qkvp.tile([P, S], BF16)
        nc.scalar.dma_start(sin_s[:], sinT.ap())
        nc.scalar.dma_start(wo_s[:], woT.ap().rearrange("(o p) n -> p o n", p=P))
        nc.scalar.dma_start(mask_s[:], maskT.ap())
        # vo ones filled once; V projection overwrites the V columns
        nc.gpsimd.memset(vo[:], 1.0)

        with tc.tile_pool(name="ppj", bufs=1, space="PSUM") as ppj:
            # PE warm-up: dummy bf16 matmuls while input DMAs stream, so the
            # HAM clock gate opens before the first real matmul.
            warm = qkvp.tile([P, 640], BF16)
            nc.vector.memset(warm[:], 0.5)
            wscr = nc.dram_tensor("warm_scratch", [1, 4], F32)
            wps = ppj.tile([P, QC], F32, tag="pp", bufs=8)
            NWARM = 24
            for _w in range(NWARM):
                nc.tensor.matmul(wps[:], warm[:, 0:P], warm[:, P:P + QC],
                                 start=(_w == 0), stop=(_w == NWARM - 1))
            wkeep = qkvp.tile([1, 4], F32)
            nc.vector.tensor_copy(wkeep[:], wps[0:1, 0:4])
            # preload the Exp activation table while the PE warms up, so the
            # first real exp of the attention phase doesn't pay the table
            # load on the critical path
            nc.scalar.activation(out=wkeep[:], in_=wkeep[:], func=EXP,
                                 scale=0.0)
            nc.sync.dma_start(wscr.ap(), wkeep[:])

            # ---- Q/K projections + RoPE.  PSUM is first copied to bf16 by
            # the (otherwise idle) ScalarE so the DVE muls run at 2x rate;
            # the 32-row partition swap goes through SBUF-SBUF DMA (engines
            # cannot cross partitions between two SBUF operands).
            with tc.tile_pool(name="rope", bufs=2) as rope:
                def rope_drain(pp0, pp1, dst, nt):
                    # drain both pairs of one seq chunk together: halves the
                    # op count, and the 4 swap DMAs move [32,2,512] each
                    # (2 KB/partition contiguous runs) split over two queues
                    qraw = rope.tile([P, 2, QC], BF16, tag="qr")
                    nc.scalar.copy(qraw[:, 0, :], pp0[:])
                    nc.scalar.copy(qraw[:, 1, :], pp1[:])
                    t1 = rope.tile([P, 2, QC], BF16, tag="t1")
                    nc.vector.tensor_mul(
                        t1[:], qraw[:],
                        cos_s[:, None, ts(nt, QC)].to_broadcast((P, 2, QC)))
                    y = rope.tile([P, 2, QC], BF16, tag="y")
                    nc.vector.tensor_mul(
                        y[:], qraw[:],
                        sin_s[:, None, ts(nt, QC)].to_broadcast((P, 2, QC)))
                    t2 = rope.tile([P, 2, QC], BF16, tag="t2")
                    for blk in range(4):
                        sb = blk ^ 1
                        eng = nc.sync if blk % 2 == 0 else nc.gpsimd
                        eng.dma_start(t2[ts(blk, 32), :, :],
                                      y[ts(sb, 32), :, :])
                    nc.vector.tensor_add(dst[:, :, ts(nt, QC)],
                                         t1[:], t2[:])

                # Q: kc-outer so compute starts as x chunks stream in
                pps = {}
                for nt in range(N_QC):
                    for pair in range(2):
                        pps[(pair, nt)] = ppj.tile(
                            [P, QC], F32, tag="pp", bufs=8,
                            name=f"pp0_{pair}_{nt}")
                for kc in range(KD):
                    for nt in range(N_QC):
                        for pair in range(2):
                            nc.tensor.matmul(
                                pps[(pair, nt)][:],
                                wq_s[:, kc, ts(pair, P)],
                                xTs[:, kc, ts(nt, QC)],
                                start=(kc == 0), stop=(kc == KD - 1),
                            )
                for nt in range(N_QC):
                    rope_drain(pps[(0, nt)], pps[(1, nt)], qt, nt)
                # K: nt-outer (x now resident) so PSUM banks are consumed
                # incrementally, chasing Q's drains
                for nt in range(N_QC):
                    ppk = {}
                    for pair in range(2):
                        ppk[pair] = ppj.tile([P, QC], F32, tag="pp", bufs=8,
                                             name=f"pp1_{pair}_{nt}")
                        for kc in range(KD):
                            nc.tensor.matmul(
                                ppk[pair][:],
                                wk_s[:, kc, ts(pair, P)],
                                xTs[:, kc, ts(nt, QC)],
                                start=(kc == 0), stop=(kc == KD - 1),
                            )
                    rope_drain(ppk[0], ppk[1], kt, nt)

        if dump:
            nc.sync.dma_start(dbg_qt.ap(), qt[:])
            nc.sync.dma_start(dbg_kt.ap(), kt[:])

        with (
            tc.tile_pool(name="att_sb", bufs=8) as esb,
            tc.tile_pool(name="att_misc", bufs=4) as misc,
            tc.tile_pool(name="pst", bufs=2, space="PSUM") as pst,
            tc.tile_pool(name="out_sb", bufs=3) as outp,
        ):
            es_g = {}
            scored = set()

            def do_score(qcv, kc):
                # scores^T for 4 heads: per pair one K=64 matmul per sub,
                # subs run concurrently in row-groups (0,*) / (64,*)
                e2 = esb.tile([P, 2, 2, QC], BF16, tag="e",
                              name=f"e{qcv}_{kc}")
                r = kc - 4 * qcv
                q0 = max(r, 0) * P          # first unmasked query column
                for pair in range(2):
                    st2 = pst.tile([P, 2, QC], F32, tag="st",
                                   name=f"st{qcv}_{kc}_{pair}")
                    for sub in range(2):
                        nc.tensor.matmul(
                            st2[:, sub, :],
                            kt[ts(sub, D_K), pair, ts(kc, P)],
                            qt[ts(sub, D_K), pair, ts(qcv, QC)],
                            start=True, stop=True,
                        )
                    if q0 > 0:
                        nc.gpsimd.memset(e2[:, pair, :, 0:q0], 0.0)
                        nc.scalar.activation(out=e2[:, pair, :, q0:QC],
                                             in_=st2[:, :, q0:QC], func=EXP,
                                             scale=0.125)
                    else:
                        nc.scalar.activation(out=e2[:, pair, :, :],
                                             in_=st2[:], func=EXP,
                                             scale=0.125)
                if r >= 0:
                    # zero the masked upper half of the diagonal 128-block
                    nc.vector.tensor_mul(
                        e2[:, :, :, ds(q0, P)], e2[:, :, :, ds(q0, P)],
                        mask_s[:, None, None, :].to_broadcast((P, 2, 2, P)))
                if dump and qcv == 0 and kc == 0:
                    nc.sync.dma_start(dbg_e2.ap(), e2[:])
                es_g[(qcv, kc)] = e2
                scored.add((qcv, kc))

            def emit_wo_sc(sc, last=False):
                # output projection for one 128-row s-chunk
                ot = outp.tile([P, D_MODEL], BF16, tag="ot", name=f"ot{sc}")
                for n2 in range(2):
                    po = pst.tile([P, 2, QC], F32, tag="st",
                                  name=f"po{sc}_{n2}")[:, 0, :]
                    for pair in range(2):
                        nc.tensor.matmul(
                            po[:],
                            att[:, pair, ts(sc, P)],
                            wo_s[:, pair, ts(n2, QC)],
                            start=(pair == 0), stop=(pair == 1),
                        )
                    if last:
                        # tail: DVE is busy normalizing; ScalarE is idle
                        nc.scalar.copy(ot[:, ts(n2, QC)], po[:])
                    else:
                        nc.vector.tensor_copy(ot[:, ts(n2, QC)], po[:])
                nc.sync.dma_start(out.ap()[ts(sc, P), :], ot[:])

            # ---- V projection into [V|1] / [1|V] layout, overlapped with
            # the first query-chunk's scores+exp stream ----
            with tc.tile_pool(name="pvp", bufs=3, space="PSUM") as pvp:
                for sc in range(N_KC):
                    pv = pvp.tile([P, HD], F32, tag="pv")
                    for kc in range(KD):
                        nc.tensor.matmul(
                            pv[:],
                            xTs[:, kc, ts(sc, P)],
                            wv_s[:, kc, :],
                            start=(kc == 0), stop=(kc == KD - 1),
                        )
                    # heads (0,2) V at cols 0:64; heads (1,3) at 64:128
                    nc.scalar.copy(
                        vo[:, sc, 0:HEADS_PER_CORE:2, 0:D_K],
                        pv[:].rearrange("p (h d) -> p h d", d=D_K)[:, 0::2, :])
                    nc.scalar.copy(
                        vo[:, sc, 1:HEADS_PER_CORE:2, D_K:P],
                        pv[:].rearrange("p (h d) -> p h d", d=D_K)[:, 1::2, :])
                    if sc == 3:
                        # scores for qc0 can start as soon as qt/kt are
                        # rotated; they only need pst banks, not pat
                        for kc2 in range(4):
                            do_score(0, kc2)
                if dump:
                    nc.sync.dma_start(dbg_vo.ap(), vo[:])

            # ---- attention + interleaved output projection ----
            with tc.tile_pool(name="pat", bufs=4, space="PSUM") as pat:
                pending_wo = None
                for qc in range(N_QC):
                    pas = {}
                    n_kc = 4 * qc + 4
                    for pair in range(2):
                        for sub in range(2):
                            pas[(pair, sub)] = pat.tile(
                                [P, QC], F32, tag="pa",
                                name=f"pa{qc}{pair}{sub}")
                    def attnv(kc_v, pas=pas, n_kc=n_kc, qc=qc):
                        e2 = es_g.pop((qc, kc_v))
                        for pair in range(2):
                            for sub in range(2):
                                h = pair * 2 + sub
                                nc.tensor.matmul(
                                    pas[(pair, sub)][:],
                                    vo[:, kc_v, h, :],
                                    e2[:, pair, sub, :],
                                    start=(kc_v == 0), stop=(kc_v == n_kc - 1),
                                )

                    KB = 3
                    done_v = 0
                    for kc in range(n_kc):
                        if (qc, kc) not in scored:
                            do_score(qc, kc)
                        if kc + 1 - done_v >= 2 * KB:
                            for kc_v in range(done_v, done_v + KB):
                                attnv(kc_v)
                            done_v += KB
                    # prefetch the next qc's first score groups + previous
                    # qc's output projection: dense PE cover for the tail
                    # attnv flush
                    if qc + 1 < N_QC:
                        for kc2 in range(4):
                            do_score(qc + 1, kc2)
                    if pending_wo is not None:
                        for sc in range(4 * pending_wo, 4 * pending_wo + 4):
                            emit_wo_sc(sc)
                        pending_wo = None
                    for kc_v in range(done_v, n_kc):
                        attnv(kc_v)

                    # normalize: att = attn_rows * recip(rowsum_rows)
                    for pair in range(2):
                        for sub in range(2):
                            pa = pas[(pair, sub)]
                            rs = misc.tile([P, QC], F32, tag="rb")
                            if sub == 0:
                                rows = slice(64, 128)   # rowsum rows
                                arows = slice(0, 64)    # attn rows
                            else:
                                rows = slice(0, 64)
                                arows = slice(64, 128)
                            # full-partition op: the custom-DVE recip
                            # misbehaves at base_partition 64; unused rows
                            # are harmless
                            nc.vector.reciprocal_approx_fast(rs[:], pa[:])
                            nc.vector.tensor_mul(
                                att[arows, pair, ts(qc, QC)],
                                pa[arows, :], rs[rows, :])
                    pending_wo = qc
                for sc in range(4 * pending_wo, 4 * pending_wo + 4):
                    emit_wo_sc(sc, last=True)
                if dump:
                    nc.sync.dma_start(dbg_att.ap(), att[:])

    nc.compile()
    return nc

_NC_CACHE = {}


def _get_nc(dump=False):
    key = ("dump" if dump else "nc")
    if key not in _NC_CACHE:
        _NC_CACHE[key] = build_nc(dump)
    return _NC_CACHE[key]


def _host_shards(x, token_positions, Wq, Wk, Wv, Wo):
    x = np.asarray(x, dtype=np.float32)
    pos = np.asarray(token_positions).astype(np.float32)
    Wq = np.asarray(Wq, dtype=np.float32)
    Wk = np.asarray(Wk, dtype=np.float32)
    Wv = np.asarray(Wv, dtype=np.float32)
    Wo = np.asarray(Wo, dtype=np.float32)

    # RoPE tables
    j = np.arange(0, D_K, 2, dtype=np.float32) / D_K
    inv_freq = (ROPE_THETA ** (-j)).astype(np.float32)        # [32]
    ang = pos[None, :] * inv_freq[:, None]                    # [32, S]
    cos32 = np.cos(ang).astype(np.float32)
    sin32 = np.sin(ang).astype(np.float32)
    cosT = np.tile(cos32, (4, 1)).astype(BF16_NP)             # [128, S]
    sinT = np.concatenate([sin32, -sin32, sin32, -sin32],
                          axis=0).astype(BF16_NP)

    # causal 128x128 block mask: mask[kp, j] = (j >= kp)
    kp = np.arange(P)[:, None]
    jq = np.arange(P)[None, :]
    mask = (jq >= kp).astype(BF16_NP)

    perm = np.concatenate([np.arange(0, D_K, 2), np.arange(1, D_K, 2)])

    in_maps = []
    for c in range(N_CORES):
        b = c // 4
        hg = c % 4
        heads = np.arange(4 * hg, 4 * hg + 4)
        rows_perm = np.concatenate([h * D_K + perm for h in heads])
        rows = np.concatenate([h * D_K + np.arange(D_K) for h in heads])
        in_maps.append({
            "xT": np.ascontiguousarray(x[b].T).astype(BF16_NP),
            "wqT": np.ascontiguousarray(Wq[rows_perm, :].T).astype(BF16_NP),
            "wkT": np.ascontiguousarray(Wk[rows_perm, :].T).astype(BF16_NP),
            "wvT": np.ascontiguousarray(Wv[rows, :].T).astype(BF16_NP),
            "woT": np.ascontiguousarray(Wo[:, rows].T).astype(BF16_NP),
            "cosT": cosT,
            "sinT": sinT,
            "maskT": mask,
        })
    return in_maps


def kernel(x, token_positions, Wq, Wk, Wv, Wo, use_f32r=True, trace=False):
    nc = _get_nc()
    in_maps = _host_shards(x, token_positions, Wq, Wk, Wv, Wo)
    res = run_bass_kernel_spmd(nc, in_maps, list(range(N_CORES)), trace=trace)
    outs = [np.asarray(res.results[c]["out"], dtype=np.float32)
            for c in range(N_CORES)]
    full = np.empty((B, S, D_MODEL), dtype=np.float32)
    for b in range(B):
        full[b] = outs[4 * b] + outs[4 * b + 1] + outs[4 * b + 2] + outs[4 * b + 3]
    kernel.last_result = res
    return full
